# revision 7
# baseline (speedup 1.0000x reference)
"""Distributed GQA attention (B=1, T=2048, D=2048, 16 Q heads / 8 KV heads,
head_dim=128, interleaved RoPE, causal) on 8 TRN2 NeuronCores.

Sharding: tensor-parallel over heads. Core c owns Q heads {2c, 2c+1} and KV
head c (exactly the GQA group). After local attention, per-(qs 512-col block)
chunks of the attention output (transposed [feat, T] layout) are AllGathered;
each core then computes a 256-column shard of the final projection with its
column slice of Wo. The host stitches the 8 column shards (transposing back).

Schedule: the four 512-wide projection waves are interleaved with attention
q-blocks at lag 1 (proj ns, then attention qs=ns-1), so AllGather chunks ship
early and spread across the kernel; the four output-projection blocks run at
the tail, covered by the AllGather pipeline. RoPE's even/odd pairing is baked
into a host-side column permutation of Wq/Wk so no PE permutation matmul is
needed (the partner lanes are fetched with two small SBUF->SBUF DMAs).
Softmax row-sums run on GpSimd/Vector (tensor_add + partition reduce +
partition_broadcast) instead of PE matmuls, keeping the PE stream free of
non-GEMM work.

Compute dtype: bf16 matmul inputs, f32 PSUM accumulation, f32 softmax stats.
x is marshalled host-side to transposed bf16 layout (pure relayout; all
arithmetic runs on device).
"""

import numpy as np

import concourse.bass as bass
import concourse.bass_isa as bass_isa
import concourse.mybir as mybir
from concourse import bacc, tile
from concourse.bass_utils import run_bass_kernel_spmd

F32 = mybir.dt.float32
BF16 = mybir.dt.bfloat16
NPBF16 = mybir.dt.np(BF16)

P = 128
T = 2048
D = 2048
NC = 8          # cores
HQ = 2          # q heads per core
DH = 128        # head dim
NT = T // P     # 16 k/t blocks
QS = 512        # q super-block width
NQS = T // QS   # 4
ND = D // P     # 16 feature blocks
SCALE = 1.0 / float(np.sqrt(DH))


def _rope_tables():
    # Half-split layout: rows 0..63 are the even features (i), rows 64..127
    # the odd partners. out[p] = q[p]*ctab[p] + q[p^64]*stab[p].
    inv_freq = 1.0 / (10000.0 ** (np.arange(0, DH, 2, dtype=np.float64) / DH))
    ang = np.arange(T, dtype=np.float64)[None, :] * inv_freq[:, None]  # [64, T]
    cos = np.cos(ang)
    sin = np.sin(ang)
    ctab = np.empty((DH, T), np.float32)
    stab = np.empty((DH, T), np.float32)
    ctab[0:64] = cos
    ctab[64:128] = cos
    stab[0:64] = -sin   # even row: out = q_e*c - q_o*s
    stab[64:128] = sin  # odd row:  out = q_o*c + q_e*s
    return ctab.astype(NPBF16), stab.astype(NPBF16)


def _trimask():
    # mask[p][tk, tq_l] = 1 if tq_l >= 128*p + tk else 0, packed [128, 4*512]
    m = np.zeros((P, 4 * QS), NPBF16)
    tk = np.arange(P)[:, None]
    tq = np.arange(QS)[None, :]
    for p in range(4):
        m[:, p * QS:(p + 1) * QS] = (tq >= p * P + tk).astype(NPBF16)
    return m


# even features first, then their odd partners (per 128-wide head block)
_EO = np.concatenate([np.arange(0, P, 2), np.arange(1, P, 2)])


def build_nc():
    nc = bacc.Bacc(num_devices=NC)

    xt_e = nc.declare_dram_parameter("xt", [D, T], BF16, isOutput=False)
    wq_e = nc.declare_dram_parameter("wq", [P, ND * HQ * DH], BF16, isOutput=False)
    wk_e = nc.declare_dram_parameter("wk", [P, ND * DH], BF16, isOutput=False)
    wv_e = nc.declare_dram_parameter("wv", [P, ND * DH], BF16, isOutput=False)
    wo_e = nc.declare_dram_parameter("wo", [P, ND * HQ * DH], BF16, isOutput=False)
    bq_e = nc.declare_dram_parameter("bq", [HQ, P], F32, isOutput=False)
    bk_e = nc.declare_dram_parameter("bk", [1, P], F32, isOutput=False)
    bv_e = nc.declare_dram_parameter("bv", [1, P], F32, isOutput=False)
    bo_e = nc.declare_dram_parameter("bo", [HQ, P], F32, isOutput=False)
    ct_e = nc.declare_dram_parameter("costab", [DH, T], BF16, isOutput=False)
    st_e = nc.declare_dram_parameter("sintab", [DH, T], BF16, isOutput=False)
    tm_e = nc.declare_dram_parameter("trimask", [P, 4 * QS], BF16, isOutput=False)
    idb_e = nc.declare_dram_parameter("identb", [P, P], BF16, isOutput=False)
    out_e = nc.declare_dram_parameter("out", [HQ * DH, T], F32, isOutput=True)

    rg = [list(range(NC))]

    with tile.TileContext(nc) as tc:
        # ---------- long-lived pools (stack order: longest-lived first) ------
        const = tc.alloc_tile_pool(name="const", bufs=1)
        identb = const.tile([P, P], BF16)
        nc.sync.dma_start(out=identb[:], in_=idb_e[:])
        trimask = const.tile([P, 4 * QS], BF16)
        warm_sb = const.tile([1, 256], BF16)
        nc.any.memset(warm_sb[:], 0.0)
        bq_t = const.tile([P, HQ], F32)
        nc.sync.dma_start(out=bq_t[:], in_=bq_e.rearrange("h p -> p h"))
        bk_t = const.tile([P, 1], F32)
        nc.sync.dma_start(out=bk_t[:], in_=bk_e.rearrange("h p -> p h"))
        bv_t = const.tile([P, 1], F32)
        nc.sync.dma_start(out=bv_t[:], in_=bv_e.rearrange("h p -> p h"))
        bo_t = const.tile([P, HQ], F32)
        nc.sync.dma_start(out=bo_t[:], in_=bo_e.rearrange("h p -> p h"))

        wpool = tc.alloc_tile_pool(name="wpool", bufs=1)
        wq_sb = wpool.tile([P, ND * HQ * DH], BF16)
        nc.sync.dma_start(out=wq_sb[:], in_=wq_e[:])
        wk_sb = wpool.tile([P, ND * DH], BF16)
        nc.sync.dma_start(out=wk_sb[:], in_=wk_e[:])
        wv_sb = wpool.tile([P, ND * DH], BF16)
        nc.sync.dma_start(out=wv_sb[:], in_=wv_e[:])
        wo_sb = wpool.tile([P, ND * HQ * DH], BF16)

        tabp = tc.alloc_tile_pool(name="tabp", bufs=1)
        ctab = tabp.tile([DH, T], BF16)
        stab = tabp.tile([DH, T], BF16)

        rope_pool = tc.alloc_tile_pool(name="ropeo", bufs=1)
        q_r = [rope_pool.tile([P, T], BF16, name=f"qr{h}") for h in range(HQ)]
        k_r = rope_pool.tile([P, T], BF16)

        vnat_pool = tc.alloc_tile_pool(name="vnat", bufs=1)
        v_nat = [vnat_pool.tile([P, DH], BF16, name=f"vnat{n}") for n in range(NT)]

        # softmax-stat + staging pools
        raccp = tc.alloc_tile_pool(name="raccp", bufs=4)
        rbp = tc.alloc_tile_pool(name="rbp", bufs=2)
        rbrp = tc.alloc_tile_pool(name="rbrp", bufs=2)
        olocp = tc.alloc_tile_pool(name="olocp", bufs=4)
        finp = tc.alloc_tile_pool(name="finp", bufs=3)
        ptpool = tc.alloc_tile_pool(name="ptpool", bufs=32)

        # proj temp pools
        qtp = tc.alloc_tile_pool(name="qtp", bufs=3)
        vtp = tc.alloc_tile_pool(name="vtp", bufs=2)
        qswp = tc.alloc_tile_pool(name="qswp", bufs=2)
        rt1p = tc.alloc_tile_pool(name="rt1p", bufs=2)
        rt2p = tc.alloc_tile_pool(name="rt2p", bufs=2)

        dram = tc.alloc_tile_pool(name="dram", bufs=1, space="DRAM")
        agin = [dram.tile([HQ * P, QS], BF16, name=f"agin{q}")
                for q in range(NQS)]
        agout = [dram.tile([NC * HQ * P, QS], BF16, name=f"agout{q}",
                           addr_space="Shared") for q in range(NQS)]
        dramw = tc.alloc_tile_pool(name="dramw", bufs=1, space="DRAM")
        warm_in = dramw.tile([1, 256], BF16, name="warmin")
        warm_out = dramw.tile([NC, 256], BF16, name="warmout",
                              addr_space="Shared")

        # warm up the CC rings early (overlaps the projection phase)
        nc.sync.dma_start(out=warm_in[:], in_=warm_sb[:])
        nc.gpsimd.collective_compute(
            "AllGather", mybir.AluOpType.bypass, replica_groups=rg,
            ins=[warm_in.opt()], outs=[warm_out.opt()])

        # ---------- phase A: load x^T (ns=0 block first so PE starts early) --
        xT_pool = tc.alloc_tile_pool(name="xT", bufs=1)
        xT = [xT_pool.tile([P, T], BF16, name=f"xT{j}") for j in range(ND)]
        for j in range(ND):
            nc.sync.dma_start(out=xT[j][:, 0:QS], in_=xt_e[j * P:(j + 1) * P, 0:QS])
        nc.sync.dma_start(out=ctab[:], in_=ct_e[:])
        nc.sync.dma_start(out=stab[:], in_=st_e[:])
        nc.sync.dma_start(out=trimask[:], in_=tm_e[:])
        for ns in range(1, NQS):
            sl = slice(ns * QS, (ns + 1) * QS)
            for j in range(ND):
                nc.sync.dma_start(out=xT[j][:, sl], in_=xt_e[j * P:(j + 1) * P, sl])
        nc.sync.dma_start(out=wo_sb[:], in_=wo_e[:])

        ag_sb = {}

        def fetch_ag(ns, ag_pool):
            for b in range(NC * HQ):
                t = ag_pool.tile([P, QS], BF16, name=f"ag{ns}_{b}")
                nc.sync.dma_start(out=t[:], in_=agout[ns][b * P:(b + 1) * P, :])
                ag_sb[(ns, b)] = t

        with tc.tile_pool(name="ppsum", bufs=2, space="PSUM") as ppsum, \
             tc.tile_pool(name="vpsum", bufs=2, space="PSUM") as vpsum, \
             tc.tile_pool(name="spsum", bufs=2, space="PSUM") as spsum, \
             tc.tile_pool(name="opsum", bufs=2, space="PSUM") as opsum:

            def proj(w_sb, m0, mw, ns):
                # PSUM <- (W^T x)[:, ns block]   (16 accumulating matmuls)
                ps = ppsum.tile([P, QS], F32, tag="pp")
                for j in range(ND):
                    nc.tensor.matmul(
                        ps[:],
                        lhsT=w_sb[:, j * mw + m0:j * mw + m0 + P],
                        rhs=xT[j][:, ns * QS:(ns + 1) * QS],
                        start=(j == 0), stop=(j == ND - 1))
                return ps

            def proj_evict(ps, bias, out_tile):
                nc.scalar.activation(
                    out=out_tile[:], in_=ps[:],
                    func=mybir.ActivationFunctionType.Identity, bias=bias)

            def rope(qt, dst, ns):
                # dst[:, sl] = qt*ctab + swap(qt)*stab  (half-split layout)
                sl = slice(ns * QS, (ns + 1) * QS)
                qsw = qswp.tile([P, QS], F32, tag="qsw")
                nc.scalar.dma_start(out=qsw[0:64, :], in_=qt[64:128, :])
                nc.scalar.dma_start(out=qsw[64:128, :], in_=qt[0:64, :])
                t1 = rt1p.tile([P, QS], F32, tag="t1")
                nc.vector.tensor_mul(t1[:], qt[:], ctab[:, sl])
                t2 = rt2p.tile([P, QS], F32, tag="t2")
                nc.vector.tensor_mul(t2[:], qsw[:], stab[:, sl])
                nc.vector.tensor_add(dst[:, sl], t1[:], t2[:])

            def proj_block(ns):
                sl = slice(ns * QS, (ns + 1) * QS)
                for h in range(HQ):
                    ps = proj(wq_sb, h * DH, HQ * DH, ns)
                    qt = qtp.tile([P, QS], F32, tag="qt", name=f"qt{ns}_{h}")
                    proj_evict(ps, bq_t[:, h:h + 1], qt)
                    rope(qt, q_r[h], ns)
                ps = proj(wk_sb, 0, DH, ns)
                kt = qtp.tile([P, QS], F32, tag="qt", name=f"kt{ns}")
                proj_evict(ps, bk_t[:, 0:1], kt)
                rope(kt, k_r, ns)
                ps = proj(wv_sb, 0, DH, ns)
                vt = vtp.tile([P, QS], BF16, tag="vt", name=f"vt{ns}")
                proj_evict(ps, bv_t[:, 0:1], vt)
                for i in range(4):
                    n = 4 * ns + i
                    vp = vpsum.tile([P, P], BF16, tag="vp", name=f"vp{n}")
                    nc.tensor.transpose(vp[:], vt[:, i * P:(i + 1) * P], identb[:])
                    nc.scalar.copy(out=v_nat[n][:], in_=vp[:])

            def attention(qs):
                qsl = slice(qs * QS, (qs + 1) * QS)
                nkb = 4 * (qs + 1)
                pts = {}
                racc = {}
                # S-pass: all S^T matmuls; exp + mask + rowsum-add trail behind
                for kb in range(nkb):
                    for h in range(HQ):
                        s_ps = spsum.tile([P, QS], F32, tag="s")
                        nc.tensor.matmul(s_ps[:],
                                         lhsT=k_r[:, kb * P:(kb + 1) * P],
                                         rhs=q_r[h][:, qsl],
                                         start=True, stop=True)
                        pt = ptpool.tile([P, QS], BF16, tag="pt",
                                         name=f"pt{qs}_{kb}_{h}")
                        nc.scalar.activation(
                            out=pt[:], in_=s_ps[:],
                            func=mybir.ActivationFunctionType.Exp, scale=SCALE)
                        ploc = kb - 4 * qs
                        if ploc >= 0:
                            nc.vector.tensor_mul(
                                pt[:], pt[:],
                                trimask[:, ploc * QS:(ploc + 1) * QS])
                        pts[(kb, h)] = pt
                        # rowsum accumulate off-PE (h0 -> gpsimd, h1 -> vector)
                        eng = nc.gpsimd if h == 0 else nc.vector
                        if kb == 1:
                            racc[h] = raccp.tile([P, QS], F32, tag="racc",
                                                 name=f"racc{qs}_{h}")
                            eng.tensor_add(racc[h][:], pts[(0, h)][:], pt[:])
                        elif kb > 1:
                            eng.tensor_add(racc[h][:], racc[h][:], pt[:])
                # AV-pass per head, then normalize + ship that head's chunk
                for h in range(HQ):
                    o_ps = opsum.tile([P, QS], F32, tag="o", name=f"ops{qs}_{h}")
                    for kb in range(nkb):
                        nc.tensor.matmul(o_ps[:], lhsT=v_nat[kb][:],
                                         rhs=pts[(kb, h)][:],
                                         start=(kb == 0), stop=(kb == nkb - 1))
                    rb = rbp.tile([P, QS], F32, tag="rb")
                    nc.gpsimd.partition_all_reduce(
                        rb[:], racc[h][:], channels=P,
                        reduce_op=bass_isa.ReduceOp.add)
                    rbr = rbrp.tile([P, QS], F32, tag="rbr")
                    nc.vector.reciprocal(rbr[:], rb[:])
                    ol = olocp.tile([P, QS], BF16, tag="ol",
                                    name=f"ol{qs}_{h}")
                    nc.vector.tensor_mul(ol[:], o_ps[:], rbr[:])
                    nc.gpsimd.dma_start(out=agin[qs][h * P:(h + 1) * P, :],
                                        in_=ol[:])
                nc.gpsimd.collective_compute(
                    "AllGather", mybir.AluOpType.bypass,
                    replica_groups=rg,
                    ins=[agin[qs].opt()], outs=[agout[qs].opt()])

            # ---- phase A2: projection waves, attention interleaved at lag 1
            for ns in range(NQS):
                proj_block(ns)
                if ns >= 1:
                    attention(ns - 1)

            # x^T no longer needed; reuse its SBUF for the gathered chunks
            xT_pool.release()
            ag_pool = tc.alloc_tile_pool(name="agsb", bufs=1)
            fetch_ag(0, ag_pool)
            fetch_ag(1, ag_pool)
            fetch_ag(2, ag_pool)
            attention(3)
            fetch_ag(3, ag_pool)

        # ---------- tail: output projection (covered by the AG pipeline) ----
        with tc.tile_pool(name="fpsum", bufs=2, space="PSUM") as fpsum:
            for ns in range(NQS):
                for m in range(HQ):
                    f_ps = fpsum.tile([P, QS], F32, tag="f",
                                      name=f"fps{ns}_{m}")
                    for b in range(NC * HQ):
                        nc.tensor.matmul(
                            f_ps[:],
                            lhsT=wo_sb[:, b * HQ * DH + m * DH:
                                       b * HQ * DH + m * DH + P],
                            rhs=ag_sb[(ns, b)][:, :],
                            start=(b == 0), stop=(b == NC * HQ - 1))
                    fin = finp.tile([P, QS], F32, tag="fin",
                                    name=f"fin{ns}_{m}")
                    nc.vector.tensor_scalar_add(fin[:], f_ps[:],
                                                bo_t[:, m:m + 1])
                    nc.sync.dma_start(
                        out=out_e[m * P:(m + 1) * P, ns * QS:(ns + 1) * QS],
                        in_=fin[:])

        ag_pool.release()
        dramw.release()
        dram.release()
        rt2p.release()
        rt1p.release()
        qswp.release()
        vtp.release()
        qtp.release()
        ptpool.release()
        finp.release()
        olocp.release()
        rbrp.release()
        rbp.release()
        raccp.release()
        vnat_pool.release()
        rope_pool.release()
        tabp.release()
        wpool.release()
        const.release()

    nc.compile()
    return nc


_NC_CACHE = None


def _get_nc():
    global _NC_CACHE
    if _NC_CACHE is None:
        _NC_CACHE = build_nc()
    return _NC_CACHE


def _warr(w):
    # [D, M] -> [P, ND*M]: row p holds feature blocks j at stride M
    m = w.shape[1]
    return np.ascontiguousarray(
        w.reshape(ND, P, m).transpose(1, 0, 2).reshape(P, ND * m)).astype(NPBF16)


def _perm_heads(w):
    # apply the even/odd half-split permutation within each 128-col head block
    nh = w.shape[1] // P
    cols = np.concatenate([h * P + _EO for h in range(nh)])
    return w[:, cols]


def _in_maps(x, Wq, bq, Wkv, bkv, Wo, bo):
    x2 = np.asarray(x, np.float32).reshape(T, D)
    xt = np.ascontiguousarray(x2.T).astype(NPBF16)
    Wq = np.asarray(Wq, np.float32)
    Wkv = np.asarray(Wkv, np.float32)
    Wo = np.asarray(Wo, np.float32)
    bq = np.asarray(bq, np.float32)
    bkv = np.asarray(bkv, np.float32)
    bo = np.asarray(bo, np.float32)
    ctab, stab = _rope_tables()
    tm = _trimask()
    identb = np.eye(P, dtype=NPBF16)
    NKV = 8
    maps = []
    for c in range(NC):
        qc = slice(HQ * DH * c, HQ * DH * (c + 1))
        kc = slice(DH * c, DH * (c + 1))
        vc = slice(NKV * DH + DH * c, NKV * DH + DH * (c + 1))
        bq_c = bq[qc].reshape(HQ, P)[:, _EO]
        bk_c = bkv[kc].reshape(1, P)[:, _EO]
        maps.append({
            "xt": xt,
            "wq": _warr(_perm_heads(Wq[:, qc])),
            "wk": _warr(_perm_heads(Wkv[:, kc])),
            "wv": _warr(Wkv[:, vc]),
            "wo": _warr(Wo[:, qc]),
            "bq": np.ascontiguousarray(bq_c),
            "bk": np.ascontiguousarray(bk_c),
            "bv": np.ascontiguousarray(bkv[vc]).reshape(1, P),
            "bo": np.ascontiguousarray(bo[qc]).reshape(HQ, P),
            "costab": ctab, "sintab": stab, "trimask": tm,
            "identb": identb,
        })
    return maps


def _assemble(results):
    full = np.empty((T, D), np.float32)
    for c in range(NC):
        full[:, HQ * DH * c:HQ * DH * (c + 1)] = results[c]["out"].T
    return full.reshape(1, T, D)


def run(trace=False, tmpdir=None, **inputs):
    nc = _get_nc()
    maps = _in_maps(**inputs)
    res = run_bass_kernel_spmd(nc, maps, core_ids=list(range(NC)), trace=trace,
                               tmpdir=tmpdir)
    return _assemble(res.results), res


def kernel(**inputs):
    out, _ = run(trace=False, **inputs)
    return out


# revision 10
# speedup vs baseline: 1.4139x; 1.4139x over previous
"""Distributed GQA attention (B=1, T=2048, D=2048, 16 Q heads / 8 KV heads,
head_dim=128, interleaved RoPE, causal) on 8 TRN2 NeuronCores.

Sharding: tensor-parallel over heads. Core c owns Q heads {2c, 2c+1} and KV
head c (exactly the GQA group). After local attention, per-(qs 512-col block)
chunks of the attention output (transposed [feat, T] layout) are AllGathered;
each core then computes a 256-column shard of the final projection with its
column slice of Wo. The host stitches the 8 column shards (transposing back).

Schedule: one long PE stream. Each projection wave ns is finely interleaved
with the S-pass of attention block qs=ns-1 (S / rowsum matmul chunks slotted
between projection groups) so the Exp activations -- the serial scalar-engine
resource -- drain at production rate without PSUM backlog. Each AV-pass runs
dense right after, ships its AllGather chunk ~2us later (softmax reciprocal +
broadcast are hoisted into the AV window), and the four output-projection
blocks run at the tail covered by the AllGather pipeline. RoPE's even/odd
pairing is baked into a host-side column permutation of Wq/Wk (no PE permute;
partner lanes come via two small SBUF->SBUF DMAs). Causally-dead columns of
diagonal S blocks are never computed (widths 512/384/256/128).

Compute dtype: bf16 matmul inputs, f32 PSUM accumulation, f32 softmax stats.
x is marshalled host-side to transposed bf16 layout (pure relayout; all
arithmetic runs on device).
"""

import numpy as np

import concourse.bass as bass
import concourse.mybir as mybir
from concourse import bacc, tile
from concourse.bass_utils import run_bass_kernel_spmd

F32 = mybir.dt.float32
BF16 = mybir.dt.bfloat16
NPBF16 = mybir.dt.np(BF16)

P = 128
T = 2048
D = 2048
NC = 8          # cores
HQ = 2          # q heads per core
DH = 128        # head dim
NT = T // P     # 16 k/t blocks
QS = 512        # q super-block width
NQS = T // QS   # 4
ND = D // P     # 16 feature blocks
SCALE = 1.0 / float(np.sqrt(DH))


def _rope_tables():
    # Half-split layout: rows 0..63 are the even features (i), rows 64..127
    # the odd partners. out[p] = q[p]*ctab[p] + q[p^64]*stab[p].
    inv_freq = 1.0 / (10000.0 ** (np.arange(0, DH, 2, dtype=np.float64) / DH))
    ang = np.arange(T, dtype=np.float64)[None, :] * inv_freq[:, None]  # [64, T]
    cos = np.cos(ang)
    sin = np.sin(ang)
    ctab = np.empty((DH, T), np.float32)
    stab = np.empty((DH, T), np.float32)
    ctab[0:64] = cos
    ctab[64:128] = cos
    stab[0:64] = -sin   # even row: out = q_e*c - q_o*s
    stab[64:128] = sin  # odd row:  out = q_o*c + q_e*s
    return ctab.astype(NPBF16), stab.astype(NPBF16)


def _trimask():
    # [128, 128] diagonal-block mask: mask[tk, ql] = 1 if ql >= tk
    tk = np.arange(P)[:, None]
    ql = np.arange(P)[None, :]
    return (ql >= tk).astype(NPBF16)


# even features first, then their odd partners (per 128-wide head block)
_EO = np.concatenate([np.arange(0, P, 2), np.arange(1, P, 2)])


def build_nc():
    nc = bacc.Bacc(num_devices=NC)

    xt_e = nc.declare_dram_parameter("xt", [D, T], BF16, isOutput=False)
    # wq is head-major: [P, HQ * ND * DH] (per-head contiguous for split DMA)
    wq_e = nc.declare_dram_parameter("wq", [P, HQ * ND * DH], BF16, isOutput=False)
    wk_e = nc.declare_dram_parameter("wk", [P, ND * DH], BF16, isOutput=False)
    wv_e = nc.declare_dram_parameter("wv", [P, ND * DH], BF16, isOutput=False)
    wo_e = nc.declare_dram_parameter("wo", [P, ND * HQ * DH], BF16, isOutput=False)
    bq_e = nc.declare_dram_parameter("bq", [HQ, P], F32, isOutput=False)
    bk_e = nc.declare_dram_parameter("bk", [1, P], F32, isOutput=False)
    bv_e = nc.declare_dram_parameter("bv", [1, P], F32, isOutput=False)
    bo_e = nc.declare_dram_parameter("bo", [HQ, P], F32, isOutput=False)
    ct_e = nc.declare_dram_parameter("costab", [DH, T], BF16, isOutput=False)
    st_e = nc.declare_dram_parameter("sintab", [DH, T], BF16, isOutput=False)
    tm_e = nc.declare_dram_parameter("trimask", [P, P], BF16, isOutput=False)
    idb_e = nc.declare_dram_parameter("identb", [P, P], BF16, isOutput=False)
    out_e = nc.declare_dram_parameter("out", [HQ * DH, T], F32, isOutput=True)

    rg = [list(range(NC))]

    with tile.TileContext(nc) as tc:
        # ---------- long-lived pools (stack order: longest-lived first) ------
        const = tc.alloc_tile_pool(name="const", bufs=1)
        identb = const.tile([P, P], BF16)
        trimask = const.tile([P, P], BF16)
        ones_col = const.tile([P, 1], BF16)
        nc.any.memset(ones_col[:], 1.0)
        warm_sb = const.tile([1, 256], BF16)
        nc.any.memset(warm_sb[:], 0.0)
        bq_t = const.tile([P, HQ], F32)
        bk_t = const.tile([P, 1], F32)
        bv_t = const.tile([P, 1], F32)
        bo_t = const.tile([P, HQ], F32)

        wpool = tc.alloc_tile_pool(name="wpool", bufs=1)
        wq_sb = wpool.tile([P, HQ * ND * DH], BF16)
        wk_sb = wpool.tile([P, ND * DH], BF16)
        wv_sb = wpool.tile([P, ND * DH], BF16)
        wo_sb = wpool.tile([P, ND * HQ * DH], BF16)

        tabp = tc.alloc_tile_pool(name="tabp", bufs=1)
        ctab = tabp.tile([DH, T], BF16)
        stab = tabp.tile([DH, T], BF16)

        rope_pool = tc.alloc_tile_pool(name="ropeo", bufs=1)
        q_r = [rope_pool.tile([P, T], BF16, name=f"qr{h}") for h in range(HQ)]
        k_r = rope_pool.tile([P, T], BF16)

        vnat_pool = tc.alloc_tile_pool(name="vnat", bufs=1)
        v_nat = [vnat_pool.tile([P, DH], BF16, name=f"vnat{n}") for n in range(NT)]

        # softmax-stat + staging pools
        rsbp = tc.alloc_tile_pool(name="rsbp", bufs=4)    # [1,QS] rowsum rows
        rbp = tc.alloc_tile_pool(name="rbp", bufs=2)      # [P,QS] bcast recip
        olocp = tc.alloc_tile_pool(name="olocp", bufs=4)
        finp = tc.alloc_tile_pool(name="finp", bufs=3)
        ptpool = tc.alloc_tile_pool(name="ptpool", bufs=32)

        # proj temp pools
        qtp = tc.alloc_tile_pool(name="qtp", bufs=3)
        vtp = tc.alloc_tile_pool(name="vtp", bufs=2)
        qswp = tc.alloc_tile_pool(name="qswp", bufs=3)
        rt1p = tc.alloc_tile_pool(name="rt1p", bufs=2)
        rt2p = tc.alloc_tile_pool(name="rt2p", bufs=2)

        dram = tc.alloc_tile_pool(name="dram", bufs=1, space="DRAM")
        agin = [dram.tile([HQ * P, QS], BF16, name=f"agin{q}")
                for q in range(NQS)]
        agout = [dram.tile([NC * HQ * P, QS], BF16, name=f"agout{q}",
                           addr_space="Shared") for q in range(NQS)]
        dramw = tc.alloc_tile_pool(name="dramw", bufs=1, space="DRAM")
        warm_in = dramw.tile([1, 256], BF16, name="warmin")
        warm_out = dramw.tile([NC, 256], BF16, name="warmout",
                              addr_space="Shared")

        # ---------- phase A DMA: first projection's inputs lead -------------
        nc.sync.dma_start(out=wq_sb[:, 0:ND * DH], in_=wq_e[:, 0:ND * DH])
        xT_pool = tc.alloc_tile_pool(name="xT", bufs=1)
        xT = [xT_pool.tile([P, T], BF16, name=f"xT{j}") for j in range(ND)]
        for j in range(ND):
            nc.sync.dma_start(out=xT[j][:, 0:QS], in_=xt_e[j * P:(j + 1) * P, 0:QS])
        nc.sync.dma_start(out=wq_sb[:, ND * DH:], in_=wq_e[:, ND * DH:])
        nc.sync.dma_start(out=wk_sb[:], in_=wk_e[:])
        nc.sync.dma_start(out=wv_sb[:], in_=wv_e[:])
        nc.sync.dma_start(out=identb[:], in_=idb_e[:])
        nc.sync.dma_start(out=bq_t[:], in_=bq_e.rearrange("h p -> p h"))
        nc.sync.dma_start(out=bk_t[:], in_=bk_e.rearrange("h p -> p h"))
        nc.sync.dma_start(out=bv_t[:], in_=bv_e.rearrange("h p -> p h"))
        nc.sync.dma_start(out=bo_t[:], in_=bo_e.rearrange("h p -> p h"))
        nc.sync.dma_start(out=ctab[:], in_=ct_e[:])
        nc.sync.dma_start(out=stab[:], in_=st_e[:])
        nc.sync.dma_start(out=trimask[:], in_=tm_e[:])
        nc.sync.dma_start(out=warm_in[:], in_=warm_sb[:])
        # warm up the CC rings early (overlaps the projection phase)
        nc.gpsimd.collective_compute(
            "AllGather", mybir.AluOpType.bypass, replica_groups=rg,
            ins=[warm_in.opt()], outs=[warm_out.opt()])
        for ns in range(1, NQS):
            sl = slice(ns * QS, (ns + 1) * QS)
            for j in range(ND):
                nc.sync.dma_start(out=xT[j][:, sl], in_=xt_e[j * P:(j + 1) * P, sl])
        nc.sync.dma_start(out=wo_sb[:], in_=wo_e[:])

        ag_sb = {}

        def fetch_ag(ns, ag_pool):
            for b in range(NC * HQ):
                t = ag_pool.tile([P, QS], BF16, name=f"ag{ns}_{b}")
                nc.sync.dma_start(out=t[:], in_=agout[ns][b * P:(b + 1) * P, :])
                ag_sb[(ns, b)] = t

        with tc.tile_pool(name="ppsum", bufs=2, space="PSUM") as ppsum, \
             tc.tile_pool(name="vpsum", bufs=1, space="PSUM") as vpsum, \
             tc.tile_pool(name="spsum", bufs=3, space="PSUM") as spsum, \
             tc.tile_pool(name="opsum", bufs=1, space="PSUM") as opsum, \
             tc.tile_pool(name="rspsum", bufs=1, space="PSUM") as rspsum:

            def rope(qt, dst, ns):
                # dst[:, sl] = qt*ctab + swap(qt)*stab  (half-split layout)
                sl = slice(ns * QS, (ns + 1) * QS)
                qsw = qswp.tile([P, QS], BF16, tag="qsw")
                nc.gpsimd.dma_start(out=qsw[0:64, :], in_=qt[64:128, :])
                nc.gpsimd.dma_start(out=qsw[64:128, :], in_=qt[0:64, :])
                t1 = rt1p.tile([P, QS], BF16, tag="t1")
                nc.vector.tensor_mul(t1[:], qt[:], ctab[:, sl])
                t2 = rt2p.tile([P, QS], BF16, tag="t2")
                nc.vector.tensor_mul(t2[:], qsw[:], stab[:, sl])
                nc.vector.tensor_add(dst[:, sl], t1[:], t2[:])

            def proj_groups(ns):
                # yields per-group emitters for the ns-th projection wave
                def qhead(h):
                    ps = ppsum.tile([P, QS], F32, tag="pp")
                    for j in range(ND):
                        nc.tensor.matmul(
                            ps[:],
                            lhsT=wq_sb[:, (h * ND + j) * DH:(h * ND + j) * DH + P],
                            rhs=xT[j][:, ns * QS:(ns + 1) * QS],
                            start=(j == 0), stop=(j == ND - 1))
                    qt = qtp.tile([P, QS], BF16, tag="qt", name=f"qt{ns}_{h}")
                    nc.scalar.activation(
                        out=qt[:], in_=ps[:],
                        func=mybir.ActivationFunctionType.Identity,
                        bias=bq_t[:, h:h + 1])
                    rope(qt, q_r[h], ns)

                def kproj():
                    ps = ppsum.tile([P, QS], F32, tag="pp")
                    for j in range(ND):
                        nc.tensor.matmul(
                            ps[:], lhsT=wk_sb[:, j * DH:j * DH + P],
                            rhs=xT[j][:, ns * QS:(ns + 1) * QS],
                            start=(j == 0), stop=(j == ND - 1))
                    kt = qtp.tile([P, QS], BF16, tag="qt", name=f"kt{ns}")
                    nc.scalar.activation(
                        out=kt[:], in_=ps[:],
                        func=mybir.ActivationFunctionType.Identity,
                        bias=bk_t[:, 0:1])
                    rope(kt, k_r, ns)

                def vproj():
                    ps = ppsum.tile([P, QS], F32, tag="pp")
                    for j in range(ND):
                        nc.tensor.matmul(
                            ps[:], lhsT=wv_sb[:, j * DH:j * DH + P],
                            rhs=xT[j][:, ns * QS:(ns + 1) * QS],
                            start=(j == 0), stop=(j == ND - 1))
                    vt = vtp.tile([P, QS], BF16, tag="vt", name=f"vt{ns}")
                    nc.scalar.activation(
                        out=vt[:], in_=ps[:],
                        func=mybir.ActivationFunctionType.Identity,
                        bias=bv_t[:, 0:1])
                    for i in range(4):
                        n = 4 * ns + i
                        vp = vpsum.tile([P, P], BF16, tag="vp", name=f"vp{n}")
                        nc.tensor.transpose(vp[:], vt[:, i * P:(i + 1) * P],
                                            identb[:])
                        nc.scalar.copy(out=v_nat[n][:], in_=vp[:])

                yield lambda: qhead(0)
                yield lambda: qhead(1)
                yield kproj
                yield vproj

            def s_chunks(qs, pts, r2_ps):
                # per-kb S + exp + mask + rowsum emitters (paced vs scalar exp)
                qbase = qs * QS
                nkb = 4 * (qs + 1)

                def chunk(kb):
                    c0 = max(0, (kb - 4 * qs)) * P if kb >= 4 * qs else 0
                    w = QS - c0
                    for h in range(HQ):
                        s_ps = spsum.tile([P, QS], F32, tag="s")
                        nc.tensor.matmul(
                            s_ps[:, c0:QS],
                            lhsT=k_r[:, kb * P:(kb + 1) * P],
                            rhs=q_r[h][:, qbase + c0:qbase + QS],
                            start=True, stop=True)
                        pt = ptpool.tile([P, QS], BF16, tag="pt",
                                         name=f"pt{qs}_{kb}_{h}")
                        nc.scalar.activation(
                            out=pt[:, c0:QS], in_=s_ps[:, c0:QS],
                            func=mybir.ActivationFunctionType.Exp, scale=SCALE)
                        if kb >= 4 * qs:
                            nc.vector.tensor_mul(pt[:, c0:c0 + P],
                                                 pt[:, c0:c0 + P], trimask[:])
                        pts[(kb, h)] = pt
                        nc.tensor.matmul(
                            r2_ps[64 * h:64 * h + 1, c0:QS],
                            lhsT=ones_col[:], rhs=pt[:, c0:QS],
                            start=(kb == 0), stop=(kb == nkb - 1),
                            skip_group_check=True)

                for kb in range(nkb):
                    yield lambda kb=kb: chunk(kb)

            def interleave(groups, chunks, chunks_first=False):
                # spread chunk emitters between the projection groups
                groups = list(groups)
                chunks = list(chunks)
                ngap = len(groups)
                done = 0
                for i, g in enumerate(groups):
                    if chunks_first:
                        take = (len(chunks) * (i + 1)) // ngap
                        while done < take:
                            chunks[done]()
                            done += 1
                        g()
                    else:
                        g()
                        take = (len(chunks) * (i + 1)) // ngap
                        while done < take:
                            chunks[done]()
                            done += 1

            def av_pass(qs, pts, r2_ps):
                nkb = 4 * (qs + 1)
                # hoist softmax stats into the AV window
                rb = {}
                for h in range(HQ):
                    r_sb = rsbp.tile([1, QS], F32, tag="rs", name=f"rs{qs}_{h}")
                    nc.scalar.copy(out=r_sb[:], in_=r2_ps[64 * h:64 * h + 1, :])
                    ri = rsbp.tile([1, QS], F32, tag="ri", name=f"ri{qs}_{h}")
                    nc.vector.reciprocal(ri[:], r_sb[:])
                    rbt = rbp.tile([P, QS], F32, tag="rb", name=f"rb{qs}_{h}")
                    nc.gpsimd.partition_broadcast(rbt[:], ri[0:1, :])
                    rb[h] = rbt
                for h in range(HQ):
                    o_ps = opsum.tile([P, QS], F32, tag="o", name=f"ops{qs}_{h}")
                    for kb in range(nkb):
                        c0 = (kb - 4 * qs) * P if kb >= 4 * qs else 0
                        nc.tensor.matmul(o_ps[:, c0:QS], lhsT=v_nat[kb][:],
                                         rhs=pts[(kb, h)][:, c0:QS],
                                         start=(kb == 0), stop=(kb == nkb - 1))
                    ol = olocp.tile([P, QS], BF16, tag="ol", name=f"ol{qs}_{h}")
                    nc.vector.tensor_mul(ol[:], o_ps[:], rb[h][:])
                    nc.gpsimd.dma_start(out=agin[qs][h * P:(h + 1) * P, :],
                                        in_=ol[:])
                nc.gpsimd.collective_compute(
                    "AllGather", mybir.AluOpType.bypass,
                    replica_groups=rg,
                    ins=[agin[qs].opt()], outs=[agout[qs].opt()])

            def fin_m(ns, m):
                f_ps = ppsum.tile([P, QS], F32, tag="pp", name=f"fps{ns}_{m}")
                for b in range(NC * HQ):
                    nc.tensor.matmul(
                        f_ps[:],
                        lhsT=wo_sb[:, b * HQ * DH + m * DH:
                                   b * HQ * DH + m * DH + P],
                        rhs=ag_sb[(ns, b)][:, :],
                        start=(b == 0), stop=(b == NC * HQ - 1))
                fin = finp.tile([P, QS], F32, tag="fin", name=f"fin{ns}_{m}")
                nc.vector.tensor_scalar_add(fin[:], f_ps[:], bo_t[:, m:m + 1])
                nc.sync.dma_start(
                    out=out_e[m * P:(m + 1) * P, ns * QS:(ns + 1) * QS],
                    in_=fin[:])

            def fin_block(ns):
                for m in range(HQ):
                    fin_m(ns, m)

            # ---- the one long PE stream ------------------------------------
            for g in proj_groups(0):
                g()
            for qs in range(NQS - 1):
                pts = {}
                r2_ps = rspsum.tile([P, QS], F32, tag="r", name=f"rps{qs}")
                interleave(proj_groups(qs + 1), s_chunks(qs, pts, r2_ps))
                av_pass(qs, pts, r2_ps)
            # release x^T SBUF; the gathered chunks reuse it
            xT_pool.release()
            ag_pool = tc.alloc_tile_pool(name="agsb", bufs=1)
            fetch_ag(0, ag_pool)
            fetch_ag(1, ag_pool)
            fetch_ag(2, ag_pool)
            # last attention block: S-pass paced against fin(0) filler
            pts = {}
            r2_ps = rspsum.tile([P, QS], F32, tag="r", name="rps3")
            interleave([lambda: fin_m(0, 0), lambda: fin_m(0, 1)],
                       s_chunks(3, pts, r2_ps), chunks_first=True)
            av_pass(3, pts, r2_ps)
            fetch_ag(3, ag_pool)
            fin_block(1)
            fin_block(2)
            fin_block(3)

        ag_pool.release()
        dramw.release()
        dram.release()
        rt2p.release()
        rt1p.release()
        qswp.release()
        vtp.release()
        qtp.release()
        ptpool.release()
        finp.release()
        olocp.release()
        rbp.release()
        rsbp.release()
        vnat_pool.release()
        rope_pool.release()
        tabp.release()
        wpool.release()
        const.release()

    nc.compile()
    return nc


_NC_CACHE = None


def _get_nc():
    global _NC_CACHE
    if _NC_CACHE is None:
        _NC_CACHE = build_nc()
    return _NC_CACHE


def _warr(w):
    # [D, M] -> [P, ND*M]: row p holds feature blocks j at stride M
    m = w.shape[1]
    return np.ascontiguousarray(
        w.reshape(ND, P, m).transpose(1, 0, 2).reshape(P, ND * m)).astype(NPBF16)


def _in_maps(x, Wq, bq, Wkv, bkv, Wo, bo):
    x2 = np.asarray(x, np.float32).reshape(T, D)
    xt = np.ascontiguousarray(x2.T).astype(NPBF16)
    Wq = np.asarray(Wq, np.float32)
    Wkv = np.asarray(Wkv, np.float32)
    Wo = np.asarray(Wo, np.float32)
    bq = np.asarray(bq, np.float32)
    bkv = np.asarray(bkv, np.float32)
    bo = np.asarray(bo, np.float32)
    ctab, stab = _rope_tables()
    tm = _trimask()
    identb = np.eye(P, dtype=NPBF16)
    NKV = 8
    maps = []
    for c in range(NC):
        qc = slice(HQ * DH * c, HQ * DH * (c + 1))
        kc = slice(DH * c, DH * (c + 1))
        vc = slice(NKV * DH + DH * c, NKV * DH + DH * (c + 1))
        # head-major, even/odd-permuted Wq: [P, HQ*ND*DH]
        wq_heads = [
            _warr(Wq[:, qc][:, h * P + _EO]) for h in range(HQ)
        ]
        bq_c = bq[qc].reshape(HQ, P)[:, _EO]
        bk_c = bkv[kc].reshape(1, P)[:, _EO]
        maps.append({
            "xt": xt,
            "wq": np.ascontiguousarray(np.concatenate(wq_heads, axis=1)),
            "wk": _warr(Wkv[:, kc][:, _EO]),
            "wv": _warr(Wkv[:, vc]),
            "wo": _warr(Wo[:, qc]),
            "bq": np.ascontiguousarray(bq_c),
            "bk": np.ascontiguousarray(bk_c),
            "bv": np.ascontiguousarray(bkv[vc]).reshape(1, P),
            "bo": np.ascontiguousarray(bo[qc]).reshape(HQ, P),
            "costab": ctab, "sintab": stab, "trimask": tm,
            "identb": identb,
        })
    return maps


def _assemble(results):
    full = np.empty((T, D), np.float32)
    for c in range(NC):
        full[:, HQ * DH * c:HQ * DH * (c + 1)] = results[c]["out"].T
    return full.reshape(1, T, D)


def run(trace=False, tmpdir=None, **inputs):
    nc = _get_nc()
    maps = _in_maps(**inputs)
    res = run_bass_kernel_spmd(nc, maps, core_ids=list(range(NC)), trace=trace,
                               tmpdir=tmpdir)
    return _assemble(res.results), res


def kernel(**inputs):
    out, _ = run(trace=False, **inputs)
    return out
